# revision 21
# baseline (speedup 1.0000x reference)
"""Trainium2 Bass kernel for BackprojectDepth.

out[b, i, y*W+x] = depth[b, 0, y, x] * (K[b,i,0]*(x+dx[b]) + K[b,i,1]*(y+dy[b]) + K[b,i,2])   for i in 0..2
out[b, 3, :]    = 1.0

Sharding: pure data parallel over batch (32 batches -> 4 per core on 8 cores).

Memory-bound; the device program minimizes wire bytes (fp16 depth in, fp16
planes 0..2 out, constant ones-plane filled host-side during the gather =>
16 MB/core instead of 40 MB) and keeps the ~360 GB/s wire busy end-to-end:

  * layout: partition p holds image rows 4p..4p+3 (depth[b] / out[b,i] are
    the plain row-major reshape [128, 4096]); outs move as half-plane
    [128, 2048] DMAs (4 KB per-partition descriptors) so the out stream
    starts early and flows smoothly.
  * planes 0/1: lin = xg*A + bias on the scalar (ACT) engine per 1024-col
    chunk (int32 x-ramp input - measured faster than fp16 - fp16 out,
    1135 ns/op), then half-plane DVE multiplies by depth (~1.4 us each).
  * plane 2: lin on the tensor engine as K=2 matmuls (stationary [1; p]
    fixed, moving [A*x + B*q + c'; 4B] host-precomputed per (b,q)) into
    4-bank PSUM tiles, drained by [128,2048] DVE multiplies (~2.2 us).

Measured engine budgets/core: ACT 36.3 us, DVE ~39 us, PE ~22 us, wire
~45 us => DMA-bound when the pipeline stays dense.

Rings: sync = xg + depth[0] + plane 0/1 outs; scalar = consts + depth
prefetches (issued up-front, before any out can block them); gpsimd SWDGE
= plane 2 outs (gpsimd does no other work, avoiding SBUF contention).
"""

import numpy as np

import concourse.tile as tile
from concourse import bacc, mybir
from concourse.bass_utils import run_bass_kernel_spmd

N_CORES = 8
B, H, W = 32, 512, 1024
HW = H * W
BPC = B // N_CORES          # batches per core
RPP = H // 128              # image rows per partition (4)
CH = RPP * W                # cols per partition in plane layout (4096)
HC = CH // 2                # half-plane cols (2048)

F32 = mybir.dt.float32
F16 = mybir.dt.float16
I32 = mybir.dt.int32

_TRACE = False              # test.py may flip this for profiling
_LAST_RESULTS = None        # BassKernelResults from the last run (for test.py)

_nc_cache = None

DEFAULT_CFG = dict(
    dpool=4, opool=4, lpool=4, psum=2,
    plane_ring=("sync", "sync", "gpsimd"),
)


def _build(**cfg_over):
    """Build + compile the per-core Bass program (SPMD: same NEFF, 8 cores)."""
    cfg = dict(DEFAULT_CFG, **cfg_over)
    nc = bacc.Bacc(
        "TRN2",
        target_bir_lowering=False,
        debug=False,
        enable_asserts=False,
        num_devices=N_CORES,
    )

    depth_d = nc.dram_tensor("depth", [BPC, H, W], F16, kind="ExternalInput")
    scale_d = nc.dram_tensor("scale", [128, BPC * 2], F32, kind="ExternalInput")
    bias_d = nc.dram_tensor("bias", [128, BPC * 2 * RPP], F32, kind="ExternalInput")
    stat_d = nc.dram_tensor("stat", [2, 128], F16, kind="ExternalInput")
    mov_d = nc.dram_tensor("mov", [2, BPC * RPP * W], F16, kind="ExternalInput")
    out_d = nc.dram_tensor("out", [BPC, 3, HW], F16, kind="ExternalOutput")

    rings = {"sync": nc.sync, "scalar": nc.scalar, "gpsimd": nc.gpsimd}

    with tile.TileContext(nc) as tc:
        with (
            tc.tile_pool(name="const", bufs=1) as cpool,
            tc.tile_pool(name="dpool", bufs=cfg["dpool"]) as dpool,
            tc.tile_pool(name="lpool", bufs=cfg["lpool"]) as lpool,
            tc.tile_pool(name="opool", bufs=cfg["opool"]) as opool,
            tc.psum_pool(name="ppool", bufs=cfg["psum"]) as ppool,
        ):
            # x-ramp on gpsimd first (ACT needs it earliest; converts i32
            # on read), then the PE consts ride the gpsimd SWDGE ring:
            # warms up its queue at t~0 (else the first plane-2 out pays
            # the ~7us cold start) and keeps the scalar ring free for pure
            # depth prefetch
            xg_t = cpool.tile([128, W], I32)
            nc.gpsimd.iota(xg_t[:], pattern=[[1, W]], base=0, channel_multiplier=0)
            stat_t = cpool.tile([2, 128], F16)
            nc.gpsimd.dma_start(stat_t[:], stat_d.ap())
            mov_t = cpool.tile([2, BPC * RPP * W], F16)
            nc.gpsimd.dma_start(mov_t[:], mov_d.ap())
            sc_t = cpool.tile([128, BPC * 2], F32)
            nc.sync.dma_start(sc_t[:], scale_d.ap())
            bi_t = cpool.tile([128, BPC * 2 * RPP], F32)
            nc.sync.dma_start(bi_t[:], bias_d.ap())

            # partition p <-> image rows 4p..4p+3 (plain row-major reshape)
            depth_ap = depth_d.ap().rearrange("b (p q) w -> b p (q w)", p=128)
            out_ap = out_d.ap().rearrange("b i (p j) -> b i p j", p=128)

            # all depth loads issued up-front so prefetch never queues
            # behind an out-DMA on the same ring; depth[0] lands in column
            # quarters split over both HWDGE rings so the first chunk
            # arrives ~2us sooner
            d_ts = []
            for b in range(BPC):
                d_t = dpool.tile([128, CH], F16)
                if b == 0:
                    for qtr, deng in enumerate((nc.sync, nc.scalar, nc.sync, nc.scalar)):
                        sl = slice(qtr * W, (qtr + 1) * W)
                        deng.dma_start(d_t[:, sl], depth_ap[b, :, sl])
                else:
                    nc.scalar.dma_start(d_t[:], depth_ap[b])
                d_ts.append(d_t)

            def act_lin(b, i):
                l_t = lpool.tile([128, CH], F16)
                col = 2 * b + i
                for q in range(RPP):
                    nc.scalar.activation(
                        l_t[:, q * W : (q + 1) * W],
                        xg_t[:],
                        mybir.ActivationFunctionType.Identity,
                        bias=bi_t[:, col * RPP + q : col * RPP + q + 1],
                        scale=sc_t[:, col : col + 1],
                    )
                return l_t

            def mul_and_store(b, i, h, o_t, lin_ap, d_t):
                sl = slice(h * HC, (h + 1) * HC)
                nc.vector.tensor_mul(o_t[:, sl], lin_ap[:, sl], d_t[:, sl])
                rings[cfg["plane_ring"][i]].dma_start(out_ap[b, i, :, sl], o_t[:, sl])

            def pe_plane(b, d_t):
                o2 = opool.tile([128, CH], F16)
                for hf in range(2):
                    ps = ppool.tile([128, HC], F32)
                    for s in range(4):
                        c0 = hf * HC + s * 512
                        q, xo = c0 // W, c0 % W
                        nc.tensor.matmul(
                            ps[:, s * 512 : (s + 1) * 512],
                            stat_t[:],
                            mov_t[:, (b * RPP + q) * W + xo : (b * RPP + q) * W + xo + 512],
                            start=True,
                            stop=True,
                        )
                    sl = slice(hf * HC, (hf + 1) * HC)
                    nc.vector.tensor_mul(o2[:, sl], ps[:], d_t[:, sl])
                    rings[cfg["plane_ring"][2]].dma_start(out_ap[b, 2, :, sl], o2[:, sl])

            def act_plane(b, i, d_t, quarters=False, defer_store=False):
                l_t = act_lin(b, i)
                o_t = opool.tile([128, CH], F16)
                if quarters:
                    # batch 0 plane 0: quarter-granularity so the first out
                    # bytes hit the wire as soon as the first depth quarter
                    # and lin chunk exist
                    for qtr in range(RPP):
                        sl = slice(qtr * W, (qtr + 1) * W)
                        nc.vector.tensor_mul(o_t[:, sl], l_t[:, sl], d_t[:, sl])
                        rings[cfg["plane_ring"][i]].dma_start(
                            out_ap[b, i, :, sl], o_t[:, sl]
                        )
                    return None
                for h in range(2):
                    sl = slice(h * HC, (h + 1) * HC)
                    nc.vector.tensor_mul(o_t[:, sl], l_t[:, sl], d_t[:, sl])
                    if not defer_store:
                        rings[cfg["plane_ring"][i]].dma_start(
                            out_ap[b, i, :, sl], o_t[:, sl]
                        )
                if defer_store:
                    # plane-1 outs ride the scalar (ACT) ring; the trigger
                    # is issued one batch later so the mul is long done and
                    # the ACT stream never blocks on it
                    return lambda: nc.scalar.dma_start(out_ap[b, 1], o_t[:])
                return None

            pending = None
            for b in range(BPC):
                d_t = d_ts[b]
                act_plane(b, 0, d_t, quarters=(b == 0))
                if pending is not None:
                    pending()
                pe_plane(b, d_t)
                pending = act_plane(b, 1, d_t, defer_store=True)
            if pending is not None:
                pending()

    nc.compile()
    return nc


def _make_in_maps(depth, inv_K, dxy):
    depth16 = np.ascontiguousarray(
        np.asarray(depth, dtype=np.float32).astype(np.float16)
    )
    K = np.asarray(inv_K, dtype=np.float64)
    dx = np.asarray(dxy, dtype=np.float64)

    # Per-batch affine coefficients: cam_i = A*x' + B*y' + C with x'=x+dx, y'=y+dy
    A = K[:, :3, 0]                                   # [B, 3]
    Bc = K[:, :3, 1]
    C = K[:, :3, 2]
    const = A * dx[:, None, 0] + Bc * dx[:, None, 1] + C   # [B, 3]

    p = np.arange(128, dtype=np.float64)
    q = np.arange(RPP, dtype=np.float64)
    x = np.arange(W, dtype=np.float64)

    # ACT path (planes 0/1): bias[b, i, q][p] = B*(4p+q) + const
    bias_all = (
        Bc[:, :2, None, None] * (4.0 * p[None, None, None, :] + q[None, None, :, None])
        + const[:, :2, None, None]
    )                                                  # [B, 2, RPP, 128]
    # PE path (plane 2): stationary rows [1, p]; moving[b, q] = [A*x + B*q + c'; 4B]
    stat = np.ascontiguousarray(
        np.stack([np.ones(128), p], axis=0).astype(np.float16)
    )
    mov0 = (
        A[:, 2, None, None] * x[None, None, :]
        + Bc[:, 2, None, None] * q[None, :, None]
        + const[:, 2, None, None]
    )                                                  # [B, RPP, W]
    mov1 = np.broadcast_to(4.0 * Bc[:, 2, None, None], mov0.shape)

    in_maps = []
    for c in range(N_CORES):
        g0 = c * BPC
        sl = slice(g0, g0 + BPC)
        bias_c = np.ascontiguousarray(
            bias_all[sl].reshape(BPC * 2 * RPP, 128).T.astype(np.float32)
        )                                              # [128, BPC*2*RPP]
        scale_c = np.ascontiguousarray(
            np.broadcast_to(
                A[sl, :2].reshape(BPC * 2).astype(np.float32), (128, BPC * 2)
            )
        )
        mov_c = np.ascontiguousarray(
            np.stack(
                [mov0[sl].reshape(-1), mov1[sl].reshape(-1)], axis=0
            ).astype(np.float16)
        )                                              # [2, BPC*RPP*W]
        in_maps.append(
            {
                "depth": depth16[sl, 0],               # [BPC, H, W] fp16
                "scale": scale_c,
                "bias": bias_c,
                "stat": stat,
                "mov": mov_c,
            }
        )
    return in_maps


def _expected_inputs(nc):
    import concourse.mybir as _mybir

    names = set()
    for alloc in nc.m.functions[0].allocations:
        if (
            isinstance(alloc, _mybir.MemoryLocationSet)
            and alloc.kind == "ExternalInput"
        ):
            names.add(alloc.memorylocations[0].name)
    return names


def _run(nc, in_maps, trace=False):
    global _LAST_RESULTS
    want = _expected_inputs(nc)
    in_maps = [{k: v for k, v in m.items() if k in want} for m in in_maps]
    res = run_bass_kernel_spmd(
        nc, in_maps, core_ids=list(range(N_CORES)), trace=trace
    )
    _LAST_RESULTS = res
    out = np.empty((B, 4, HW), dtype=np.float32)
    for c in range(N_CORES):
        out[c * BPC : (c + 1) * BPC, :3] = res.results[c]["out"]  # fp16 -> f32
    out[:, 3, :] = 1.0
    return out


def kernel(depth, inv_K, dxy):
    global _nc_cache
    in_maps = _make_in_maps(depth, inv_K, dxy)
    if _nc_cache is None:
        _nc_cache = _build()
    return _run(_nc_cache, in_maps, trace=_TRACE)
